# revision 3
# baseline (speedup 1.0000x reference)
"""Fused OOQKV attention-with-generated-transform kernel for Trainium2 (v5).

Math (per head h, one head per core):
  g = gelu(x @ Wg_h + bg_h)            # [T, 64, 64] per-token transform
  q,k,v = x @ W{q,k,v}_h + b           # [T, 64]
  qg[t] = q[t] @ g[t]
  att = softmax(qg @ k^T)              # per batch, no scaling
  out_h = att @ v

Key layout/engine choices (all measured on HW):
  - All matmul inputs fp16 (1 cycle/row streaming like f32r, half DMA,
    cheap LDWEIGHTS).  PSUM accumulation stays f32.
  - Wg columns host-permuted so each 512-chunk is (e-major, dsub-minor):
    gelu writes contiguously; chunk pairs share a [128,1024] 2-bank PSUM
    tile so one ACT instruction covers 1024 elems.
  - g-bias: 3 pairs via K=1 ones-matmuls on PE, 1 pair via a DVE
    tensor-add against a broadcast bias tile (balances PE vs DVE load).
  - q/v biases ride the PSUM->SBUF extraction adds on DVE (no bias
    matmul, no ones column in the projection; v's softmax-denominator
    ones column is memset once).
  - qg contraction on DVE in packed fp16 (2x mode): per pair one mult
    against a strided q broadcast view, add-tree, halves-add, one reduce.
  - k is produced TRANSPOSED by stationary-Wk matmuls (bias via ACT
    per-partition bias) grouped late in phase 1; batch 0/1 score bursts
    also interleave there so the attention tail is short.
  - Phase 2 pipelines S^T/exp/AV across batches; es and v are bf16 so
    three batches of staged exp() fit in SBUF.
  - Few, large DMAs: the DMA-trigger rate (~1us/trigger on a sequencer)
    dominated startup with many small transfers.
"""

import sys

sys.path.insert(0, "/opt/trn_rl_repo")

import numpy as np

B, N, E, H, D = 4, 1024, 512, 8, 64
T = B * N                 # 4096 flattened tokens
NTT = T // 128            # 32 token tiles
NKT = E // 128            # 4 contraction tiles
PKW = 3 * D               # packed q|v|k weight width per kt
M = 8                     # cores

_cache = {}


def _build():
    if "nc" in _cache:
        return _cache["nc"]
    from contextlib import ExitStack

    import concourse.bass as bass
    import concourse.bacc as bacc
    import concourse.mybir as mybir
    import concourse.tile as tile
    from concourse.masks import make_identity

    F32 = mybir.dt.float32
    F32R = mybir.dt.float32r
    F16 = mybir.dt.float16
    BF16 = mybir.dt.bfloat16
    AF = mybir.ActivationFunctionType
    ALU = mybir.AluOpType
    AX = mybir.AxisListType

    nc = bacc.Bacc(trn_type="TRN2")
    xT_d = nc.dram_tensor("xT", [E, T], F16, kind="ExternalInput")
    Wg_d = nc.dram_tensor("Wg", [E, D * D], F16, kind="ExternalInput")
    # [128, 4*192]: per-partition packed (Wq | Wv | Wk) for each kt
    Wp_d = nc.dram_tensor("Wp", [128, NKT * PKW], F16, kind="ExternalInput")
    # [1, 128 + 4096]: (bq | bv) | bg(permuted)
    bp_d = nc.dram_tensor("bp", [1, 2 * D + D * D], F16, kind="ExternalInput")
    bk_d = nc.dram_tensor("bk", [D, 1], F32, kind="ExternalInput")
    outT_d = nc.dram_tensor("outT", [D + 1, T], F32, kind="ExternalOutput")

    with tile.TileContext(nc) as tc, ExitStack() as ctx:
        const = ctx.enter_context(tc.tile_pool(name="const", bufs=1))
        acts = ctx.enter_context(tc.tile_pool(name="acts", bufs=1))
        q_pool = ctx.enter_context(tc.tile_pool(name="qp", bufs=10))
        gpool = ctx.enter_context(tc.tile_pool(name="gp", bufs=4))
        zpool = ctx.enter_context(tc.tile_pool(name="zp", bufs=2))
        apool = ctx.enter_context(tc.tile_pool(name="ap", bufs=9))
        ppool = ctx.enter_context(tc.tile_pool(name="pp", bufs=3))
        hpool = ctx.enter_context(tc.tile_pool(name="hp", bufs=10))
        espool = ctx.enter_context(tc.tile_pool(name="es", bufs=16))
        outp = ctx.enter_context(tc.tile_pool(name="outp", bufs=4))
        pairs = ctx.enter_context(
            tc.tile_pool(name="pgp", bufs=3, space="PSUM"))
        pqtr = ctx.enter_context(
            tc.tile_pool(name="pqtr", bufs=1, space="PSUM"))

        # ---- constants / weights (3 packed DMAs on the sync queue) ----
        wp_sb = const.tile([128, NKT * PKW], F16)
        nc.sync.dma_start(wp_sb[:], Wp_d[:, :])
        bp_sb = const.tile([1, 2 * D + D * D], F16)
        nc.sync.dma_start(bp_sb[:], bp_d[:, :])
        bk_sb = const.tile([D, 1], F32)
        nc.sync.dma_start(bk_sb[:], bk_d[:, :])
        wqv_sb = [wp_sb[:, kt * PKW:kt * PKW + 2 * D] for kt in range(NKT)]
        wk_sb = [wp_sb[:, kt * PKW + 2 * D:(kt + 1) * PKW]
                 for kt in range(NKT)]
        bg_sb = bp_sb[:, 2 * D:]
        ones32 = const.tile([1, 128], F32)
        nc.vector.memset(ones32[:], 1.0)
        ones16 = const.tile([1, 128], F16)
        nc.vector.tensor_copy(ones16[:], ones32[:])
        ident = const.tile([128, 128], F16)
        make_identity(nc, ident[:])

        # broadcast bias tiles (one-time, via ones-matmuls through PSUM)
        qvb_bc = const.tile([128, 2 * D], F32)
        gb3_bc = const.tile([128, 1024], F32)
        binit = pairs.tile([128, 1024], F32, tag="pgpair", name="binit")
        nc.tensor.matmul(binit[:, 0:2 * D], ones16[:], bp_sb[:, 0:2 * D],
                         start=True, stop=True)
        nc.vector.tensor_copy(qvb_bc[:], binit[:, 0:2 * D])
        binit2 = pairs.tile([128, 1024], F32, tag="pgpair", name="binit2")
        for half in range(2):
            nc.tensor.matmul(binit2[:, half * 512:half * 512 + 512],
                             ones16[:], bg_sb[:, 3072 + half * 512:
                                              3072 + half * 512 + 512],
                             start=True, stop=True)
        nc.vector.tensor_copy(gb3_bc[:], binit2[:])

        # ---- resident xT: chunk 0 (tiles 0-3), then the rest ----
        xt_sb = []
        for kt in range(NKT):
            xt = acts.tile([128, T], F16, tag=f"xt{kt}", name=f"xt{kt}")
            xt_sb.append(xt)
        for kt in range(NKT):
            nc.gpsimd.dma_start(xt_sb[kt][:, 0:512],
                                xT_d[kt * 128:(kt + 1) * 128, 0:512])
        for kt in range(NKT):
            nc.sync.dma_start(xt_sb[kt][:, 512:1024],
                              xT_d[kt * 128:(kt + 1) * 128, 512:1024])

        # ---- resident Wg: 8 half-transfers on the scalar queue ----
        wg_sb = []
        for kt in range(NKT):
            wgt = acts.tile([128, D * D], F16, tag=f"wg{kt}", name=f"wg{kt}")
            wg_sb.append(wgt)
        def wg_eng(q, kt):
            if kt in (0, 1):
                return nc.scalar
            if kt == 2:
                return nc.gpsimd if q < 2 else nc.sync
            return nc.gpsimd
        for q in range(4):
            for kt in range(NKT):
                wg_eng(q, kt).dma_start(
                    wg_sb[kt][:, q * 1024:(q + 1) * 1024],
                    Wg_d[kt * 128:(kt + 1) * 128, q * 1024:(q + 1) * 1024])

        # remaining xT on the gpsimd trigger queue
        for kt in range(NKT):
            nc.gpsimd.dma_start(xt_sb[kt][:, 1024:T],
                                xT_d[kt * 128:(kt + 1) * 128, 1024:T])

        # ---- persistent per-head activations ----
        v_sb = acts.tile([128, NTT, D + 1], BF16)
        vw = v_sb[:]
        vones = bass.AP(tensor=vw.tensor, offset=vw.offset + D,
                        ap=[vw.ap[0], [D + 1, NTT]])
        nc.gpsimd.memset(vones, 1.0)  # softmax-denominator ones column
        kT_sb = acts.tile([D, T], F16)
        qgT_sb = acts.tile([D, T], F16)

        NMT = N // 128  # m tiles per batch
        pending = []    # (tc0, qg_t) awaiting PE transpose + copy
        state = {}      # tt -> dict(q_t, gts, acc)
        es_all = {}     # b -> list of es pair tiles

        def flush_pending(upto):
            while len(pending) > upto:
                ptc0, pqg = pending.pop(0)
                ptr = pqtr.tile([D, 128], F16, tag="tr", name="ptr")
                nc.tensor.transpose(ptr[:], pqg[:], ident[:])
                nc.vector.tensor_copy(qgT_sb[:, ptc0:ptc0 + 128], ptr[:])

        def mult_add(st, j):
            # DVE: P_j = g~_j * q[:, 16j:16j+16] (bcast view); acc += P_j
            qw = st["q_t"][:]
            q3 = bass.AP(tensor=qw.tensor, offset=qw.offset + j * 16,
                         ap=[qw.ap[0], [8, 2], [0, D], [1, 8]])
            gv = st["gts"][j][:].rearrange("p (h e d) -> p h e d", h=2, d=8)
            with nc.allow_low_precision(reason="fp16 qg accumulation"):
                if j == 0:
                    nc.vector.tensor_tensor(
                        st["acc"][:].rearrange("p (h e d) -> p h e d",
                                               h=2, d=8),
                        gv, q3, op=ALU.mult)
                else:
                    prod = ppool.tile([128, 1024], F16, tag="prod",
                                      name="prod")
                    nc.vector.tensor_tensor(
                        prod[:].rearrange("p (h e d) -> p h e d", h=2, d=8),
                        gv, q3, op=ALU.mult)
                    nc.vector.tensor_tensor(st["acc"][:], st["acc"][:],
                                            prod[:], op=ALU.add)

        def stage_a(tt):
            tc0 = tt * 128
            xs = [xt_sb[kt][:, tc0:tc0 + 128] for kt in range(NKT)]
            st = state[tt] = {}
            pq = pqtr.tile([128, 2 * D], F32, tag="pq", name="pq")
            prs = []
            for j in range(2):
                pr = pairs.tile([128, 1024], F32, tag="pgpair",
                                name=f"prA{j}")
                prs.append(pr)
            for kt in range(NKT):
                nc.tensor.matmul(pq[:], xs[kt], wqv_sb[kt],
                                 start=(kt == 0), stop=(kt == NKT - 1))
                for j in range(2):
                    for half in range(2):
                        c0 = j * 1024 + half * 512
                        nc.tensor.matmul(
                            prs[j][:, half * 512:half * 512 + 512],
                            xs[kt], wg_sb[kt][:, c0:c0 + 512],
                            start=(kt == 0), stop=False)
            for j in range(2):
                for half in range(2):
                    c0 = j * 1024 + half * 512
                    nc.tensor.matmul(
                        prs[j][:, half * 512:half * 512 + 512],
                        ones16[:], bg_sb[:, c0:c0 + 512],
                        start=False, stop=True)
            flush_pending(1)
            gts = []
            for j in range(2):
                g_t = gpool.tile([128, 1024], F16, tag="g", name=f"gA{j}")
                nc.scalar.activation(g_t[:], prs[j][:], AF.Gelu)
                gts.append(g_t)
            st["gts"] = gts
            # q/v extraction with fused bias adds (DVE)
            q_t = q_pool.tile([128, D], F16, tag="q", name="q_t")
            with nc.allow_low_precision(reason="fp16 q"):
                nc.vector.tensor_tensor(q_t[:], pq[:, 0:D], qvb_bc[:, 0:D],
                                        op=ALU.add)
                nc.vector.tensor_tensor(v_sb[:, tt, 0:D], pq[:, D:2 * D],
                                        qvb_bc[:, D:2 * D], op=ALU.add)
            st["q_t"] = q_t
            st["acc"] = apool.tile([128, 1024], F16, tag="acc", name="acc")
            mult_add(st, 0)
            mult_add(st, 1)

        def stage_b(tt):
            tc0 = tt * 128
            xs = [xt_sb[kt][:, tc0:tc0 + 128] for kt in range(NKT)]
            st = state[tt]
            prs = []
            for j in range(2):
                pr = pairs.tile([128, 1024], F32, tag="pgpair",
                                name=f"prB{j}")
                prs.append(pr)
            for kt in range(NKT):
                for j in range(2):
                    for half in range(2):
                        c0 = 2048 + j * 1024 + half * 512
                        nc.tensor.matmul(
                            prs[j][:, half * 512:half * 512 + 512],
                            xs[kt], wg_sb[kt][:, c0:c0 + 512],
                            start=(kt == 0),
                            stop=(j == 1 and kt == NKT - 1))
            # pair B0 bias on PE; pair B1 bias on DVE via broadcast add
            for half in range(2):
                c0 = 2048 + half * 512
                nc.tensor.matmul(prs[0][:, half * 512:half * 512 + 512],
                                 ones16[:], bg_sb[:, c0:c0 + 512],
                                 start=False, stop=True)
            g_t0 = gpool.tile([128, 1024], F16, tag="g", name="gB0")
            nc.scalar.activation(g_t0[:], prs[0][:], AF.Gelu)
            st["gts"].append(g_t0)
            z_t = zpool.tile([128, 1024], F16, tag="z", name="z_t")
            with nc.allow_low_precision(reason="fp16 z"):
                nc.vector.tensor_tensor(z_t[:], prs[1][:], gb3_bc[:],
                                        op=ALU.add)
            g_t1 = gpool.tile([128, 1024], F16, tag="g", name="gB1")
            nc.scalar.activation(g_t1[:], z_t[:], AF.Gelu)
            st["gts"].append(g_t1)
            mult_add(st, 2)
            mult_add(st, 3)
            with nc.allow_low_precision(reason="fp16 qg accumulation"):
                accH = hpool.tile([128, 512], F16, tag="accH", name="accH")
                nc.vector.tensor_tensor(accH[:], st["acc"][:, 0:512],
                                        st["acc"][:, 512:1024], op=ALU.add)
                qg_t = hpool.tile([128, D], F16, tag="qg", name="qg_t")
                nc.vector.tensor_reduce(
                    qg_t[:], accH[:].rearrange("p (e d) -> p e d", d=8),
                    axis=AX.X, op=ALU.add)
            pending.append((tc0, qg_t))
            del state[tt]["gts"]

        def k_pass():
            # kT via stationary Wk; identities grouped (one table switch)
            kp_tiles = []
            for c in range(8):
                kp = pairs.tile([128, 1024], F32, tag="pgpair",
                                name=f"kp{c % 2}")
                for kt in range(NKT):
                    nc.tensor.matmul(kp[0:D, 0:512], wk_sb[kt],
                                     xt_sb[kt][:, c * 512:(c + 1) * 512],
                                     start=(kt == 0), stop=(kt == NKT - 1))
                nc.scalar.activation(kT_sb[:, c * 512:(c + 1) * 512],
                                     kp[0:D, 0:512], AF.Identity,
                                     bias=bk_sb[:])
                kp_tiles.append(kp)

        def s_burst(b):
            esl = es_all[b] = []
            for mt in range(NMT):
                mc0 = b * N + mt * 128
                ps_ = pairs.tile([128, 1024], F32, tag="pgpair", name="ps_")
                for nch in range(2):
                    nc.tensor.matmul(
                        ps_[:, nch * 512:nch * 512 + 512],
                        kT_sb[:, mc0:mc0 + 128],
                        qgT_sb[:, b * N + nch * 512:b * N + nch * 512 + 512],
                        start=True, stop=True)
                e_t = espool.tile([128, 1024], BF16, tag="es", name="e_t")
                nc.scalar.activation(e_t[:], ps_[:], AF.Exp)
                esl.append(e_t)

        def av_burst(b):
            esl = es_all.pop(b)
            for nch in range(2):
                pav = pairs.tile([128, 1024], F32, tag="pgpair", name="pav")
                for mt in range(NMT):
                    nc.tensor.matmul(
                        pav[0:D + 1, 0:512], v_sb[:, b * NMT + mt, :],
                        esl[mt][:, nch * 512:nch * 512 + 512],
                        start=(mt == 0), stop=(mt == NMT - 1))
                nc0 = b * N + nch * 512
                o_t = outp.tile([D + 1, 512], F32, tag="o", name="o_t")
                nc.vector.tensor_copy(o_t[:], pav[0:D + 1, 0:512])
                nc.sync.dma_start(outT_d[:, nc0:nc0 + 512], o_t[:])

        for tt in range(8):
            stage_a(tt)
        for tt in range(8):
            stage_b(tt)
        for tt in range(8, NTT):
            stage_a(tt)
            stage_b(tt)
            if tt == 25:
                k_pass()
            elif tt == 27:
                s_burst(0)
            elif tt == 29:
                s_burst(1)
            elif tt == 30:
                av_burst(0)
            elif tt == 31:
                s_burst(2)
        flush_pending(0)
        av_burst(1)
        s_burst(3)
        av_burst(2)
        av_burst(3)

    nc.compile()
    _cache["nc"] = nc
    return nc


def _make_in_maps(x, Wq, bq, Wk, bk, Wv, bv, Wg, bg):
    x = np.asarray(x, dtype=np.float32)
    xT = np.ascontiguousarray(x.reshape(T, E).T.astype(np.float16))
    in_maps = []
    for h in range(M):
        c0 = h * D
        # Wg columns permuted: chunk oc of 512 cols reordered (e, dsub)
        Wgh = np.asarray(Wg[:, h * D * D:(h + 1) * D * D], dtype=np.float32)
        Wgp = np.ascontiguousarray(
            Wgh.reshape(E, 8, 8, D).transpose(0, 1, 3, 2).reshape(E, D * D)
            .astype(np.float16))
        bgh = np.asarray(bg[h * D * D:(h + 1) * D * D], dtype=np.float32)
        bgp = (bgh.reshape(8, 8, D).transpose(0, 2, 1).reshape(D * D)
               .astype(np.float16))
        # packed per-kt (Wq | Wv | Wk): [128, 4*192]
        Wp = np.zeros((128, NKT * PKW), dtype=np.float16)
        for kt in range(NKT):
            r0 = kt * 128
            Wp[:, kt * PKW:kt * PKW + D] = Wq[r0:r0 + 128, c0:c0 + D]
            Wp[:, kt * PKW + D:kt * PKW + 2 * D] = Wv[r0:r0 + 128,
                                                      c0:c0 + D]
            Wp[:, kt * PKW + 2 * D:(kt + 1) * PKW] = Wk[r0:r0 + 128,
                                                        c0:c0 + D]
        bp = np.zeros((1, 2 * D + D * D), dtype=np.float16)
        bp[0, 0:D] = bq[c0:c0 + D]
        bp[0, D:2 * D] = bv[c0:c0 + D]
        bp[0, 2 * D:] = bgp
        in_maps.append(dict(
            xT=xT,
            Wg=Wgp,
            Wp=Wp,
            bp=bp,
            bk=np.ascontiguousarray(
                np.asarray(bk[c0:c0 + D], np.float32).reshape(D, 1)),
        ))
    return in_maps


def kernel(x, Wq, bq, Wk, bk, Wv, bv, Wg, bg):
    from concourse import bass_utils

    nc = _build()
    in_maps = _make_in_maps(x, Wq, bq, Wk, bk, Wv, bv, Wg, bg)
    _cache["in_maps"] = in_maps
    res = bass_utils.run_bass_kernel_spmd(nc, in_maps, core_ids=list(range(M)))
    out = np.empty((B, N, H, D), dtype=np.float32)
    for h in range(M):
        oT = res.results[h]["outT"]           # [65, T]
        o = (oT[:D] / oT[D:D + 1]).T          # [T, 64]
        out[:, :, h, :] = o.reshape(B, N, D)
    return out.reshape(B, N, E)
